# revision 1
# baseline (speedup 1.0000x reference)
"""CAB multi-head attention on 8 Trainium2 NeuronCores.

Sharding: fully data-parallel, core c -> (batch b = c//2, query-half = c%2).
Each core computes 256 query rows against all 512 keys of its batch.
No collectives. Host does transposes/packing; device does all FLOPs.

Per-core layout conventions (features on partitions, tokens on free):
  QT/KT [E, t] f32r; V [s, e] f32r; scoresT/attnT [s, t] (softmax along
  partitions via one-hot-column matmuls, no max subtraction needed);
  CAB pairs i-major: h/h2 [(d, i%2), j]; comp [(iic, i%2+h), j] is
  PE-transposed into biasT [j, (tt, jc, c)] and pre-loaded into the
  scores PSUM via an identity matmul with a strided moving AP.
"""
import sys

sys.path.insert(0, "/opt/trn_rl_repo")

import numpy as np
import ml_dtypes
from contextlib import ExitStack

import concourse.bacc as bacc
import concourse.tile as tile
from concourse import mybir
from concourse.bass_utils import run_bass_kernel_spmd

F32 = mybir.dt.float32
F32R = mybir.dt.float32r
BF16 = mybir.dt.bfloat16
AF = mybir.ActivationFunctionType
ALU = mybir.AluOpType

B, N, E, H, SD, HID = 4, 512, 1024, 16, 64, 64
D = E // H
NQ = 256            # query rows per core
NCORES = 8
NTT = NQ // 8       # 32 tt groups (4 i-pairs each) in the CAB stage

_BF = ml_dtypes.bfloat16


def _build_program(debug=False):
    nc = bacc.Bacc("TRN2", target_bir_lowering=False, debug=False,
                   num_devices=NCORES)

    def din(name, shape, dt):
        return nc.dram_tensor(name, list(shape), dt, kind="ExternalInput").ap()

    d = {}
    d["qT"] = din("qT", (E, NQ), F32R)
    d["kT"] = din("kT", (E, N), F32R)
    d["vT"] = din("vT", (E, N), F32R)
    d["seT"] = din("seT", (SD, N), F32R)
    d["seQ"] = din("seQ", (SD, NQ), F32R)
    d["wq"] = din("wq", (E, E), F32R)
    d["wk"] = din("wk", (E, E), F32R)
    d["wv"] = din("wv", (E, E), F32R)
    d["wo"] = din("wo", (E, E), BF16)
    d["w1a"] = din("w1a", (SD, 128), F32R)
    d["w1b"] = din("w1b", (SD, 128), F32R)
    d["w2bd"] = din("w2bd", (128, 128), BF16)
    d["w3bd"] = din("w3bd", (128, 32), BF16)
    d["id128"] = din("id128", (128, 128), BF16)
    d["hsel"] = din("hsel", (128, H * 16), BF16)
    d["bq128"] = din("bq128", (128, 8), F32)
    d["bk128"] = din("bk128", (128, 8), F32)
    d["b1d"] = din("b1d", (128, 1), F32)
    d["b2d"] = din("b2d", (128, 1), F32)
    d["t128"] = din("t128", (128, 1), F32)
    d["b3t"] = din("b3t", (128, 1), F32)
    d["bv2d"] = din("bv2d", (1, E), F32R)
    d["ones1"] = din("ones1", (1, 128), F32R)
    out_d = nc.dram_tensor("out", [NQ, E], F32, kind="ExternalOutput").ap()
    rscratch = nc.dram_tensor("rscratch", [16, NQ], F32).ap()
    dbg = {}
    if debug:
        dbg["dQT"] = nc.dram_tensor("dQT", [128, NQ], F32, kind="ExternalOutput").ap()
        dbg["dKT"] = nc.dram_tensor("dKT", [128, N], F32, kind="ExternalOutput").ap()
        dbg["dV"] = nc.dram_tensor("dV", [128, 512], BF16, kind="ExternalOutput").ap()
        dbg["dhjT"] = nc.dram_tensor("dhjT", [128, N], BF16, kind="ExternalOutput").ap()
        dbg["dhiT"] = nc.dram_tensor("dhiT", [128, 128], F32, kind="ExternalOutput").ap()
        dbg["dbiasT"] = nc.dram_tensor("dbiasT", [128, NTT * 512], BF16, kind="ExternalOutput").ap()
        dbg["dat"] = nc.dram_tensor("dat", [128, NQ], BF16, kind="ExternalOutput").ap()
        dbg["dsums"] = nc.dram_tensor("dsums", [16, NQ], F32, kind="ExternalOutput").ap()
        dbg["davU"] = nc.dram_tensor("davU", [128, NQ], F32, kind="ExternalOutput").ap()
        dbg["davN"] = nc.dram_tensor("davN", [128, NQ], BF16, kind="ExternalOutput").ap()

    with tile.TileContext(nc) as tc, ExitStack() as ctx:
        # ---------------- persistent SBUF pools ----------------
        cst = ctx.enter_context(tc.tile_pool(name="cst", bufs=1))
        big = ctx.enter_context(tc.tile_pool(name="big", bufs=1))

        def cload(name, shape, dt):
            t = cst.tile(list(shape), dt, tag=name, name=name)
            nc.sync.dma_start(t[:], d[name][:])
            return t

        id128 = cload("id128", (128, 128), BF16)
        hsel = cload("hsel", (128, H * 16), BF16)
        w1a = cload("w1a", (SD, 128), F32R)
        w1b = cload("w1b", (SD, 128), F32R)
        w2bd = cload("w2bd", (128, 128), BF16)
        w3bd = cload("w3bd", (128, 32), BF16)
        bq128 = cload("bq128", (128, 8), F32)
        bk128 = cload("bk128", (128, 8), F32)
        b1d = cload("b1d", (128, 1), F32)
        b2d = cload("b2d", (128, 1), F32)
        t128 = cload("t128", (128, 1), F32)
        b3t = cload("b3t", (128, 1), F32)
        bv2d = cload("bv2d", (1, E), F32R)
        ones1 = cload("ones1", (1, 128), F32R)
        seT = cload("seT", (SD, N), F32R)
        seQ = cload("seQ", (SD, NQ), F32R)

        # resident per-core inputs, chunked on k (one DMA each, k-chunk kc
        # of a [E, t] tensor lives in tile kc as [128, t])
        def kchunks(name, t, dt, ntile=8):
            ts = []
            for k in range(ntile):
                tt = big.tile([128, t], dt, tag=f"{name}{k}", name=f"{name}{k}")
                nc.sync.dma_start(tt[:], d[name][k * 128:(k + 1) * 128, :])
                ts.append(tt)
            return ts

        qTt = kchunks("qT", NQ, F32R)
        kTt = kchunks("kT", N, F32R)
        # Wv rows resident (rhs of V-proj), Wo rows resident (rhs of out-proj)
        wv_r = kchunks("wv", E, F32R)
        wo_r = kchunks("wo", E, BF16)

        # persistent intermediates
        QT = [big.tile([128, NQ], F32R, tag=f"QT{k}", name=f"QT{k}") for k in range(8)]
        KT = [big.tile([128, N], F32R, tag=f"KT{k}", name=f"KT{k}") for k in range(8)]
        Vsb = [[big.tile([128, 512], BF16, tag=f"V{st}_{et}", name=f"V{st}_{et}")
                for et in range(2)] for st in range(4)]
        hjT = big.tile([128, N], BF16, tag="hjT")
        hiT = big.tile([128, 128], F32, tag="hiT")
        biasT = big.tile([128, NTT * 512], BF16, tag="biasT")
        avU = [big.tile([128, NQ], F32, tag=f"avU{hp}", name=f"avU{hp}") for hp in range(8)]
        avN = [big.tile([128, NQ], BF16, tag=f"avN{hp}", name=f"avN{hp}") for hp in range(8)]
        sums_sb = big.tile([16, NQ], F32, tag="sums_sb")
        recip_sb = big.tile([16, NQ], F32, tag="recip_sb")

        # ---------------- phase 1: projections + W1 ----------------
        with tc.tile_pool(name="wcol", bufs=3) as wcol, \
             tc.tile_pool(name="p1ps", bufs=3, space="PSUM") as p1ps, \
             tc.tile_pool(name="w1ps", bufs=1, space="PSUM") as w1ps:

            # Q/K projections: out-chunk ec outer, contraction kc inner.
            for ec in range(8):
                wq_c = wcol.tile([128, 1024], F32R, tag="wcol")
                nc.sync.dma_start(
                    wq_c[:],
                    d["wq"][:, ec * 128:(ec + 1) * 128]
                    .rearrange("(k p) c -> p k c", p=128))
                ps = p1ps.tile([128, 512], F32, tag="p1", name="qps")[:, 0:NQ]
                for kc in range(8):
                    nc.tensor.matmul(ps[:], wq_c[:, kc * 128:(kc + 1) * 128],
                                     qTt[kc][:], start=(kc == 0),
                                     stop=(kc == 7))
                nc.vector.tensor_scalar(QT[ec][:], ps[:],
                                        bq128[:, ec:ec + 1], None, ALU.add)

            for ec in range(8):
                wk_c = wcol.tile([128, 1024], F32R, tag="wcol")
                nc.sync.dma_start(
                    wk_c[:],
                    d["wk"][:, ec * 128:(ec + 1) * 128]
                    .rearrange("(k p) c -> p k c", p=128))
                ps = p1ps.tile([128, 512], F32, tag="p1", name="kvps")
                for kc in range(8):
                    nc.tensor.matmul(ps[:], wk_c[:, kc * 128:(kc + 1) * 128],
                                     kTt[kc][:], start=(kc == 0),
                                     stop=(kc == 7))
                nc.vector.tensor_scalar(KT[ec][:], ps[:],
                                        bk128[:, ec:ec + 1], None, ALU.add)

            # V projection: V[s, e] tiles; lhsT = vT column-blocks.
            for st in range(4):
                vt_c = wcol.tile([128, 1024], F32R, tag="wcol")
                nc.sync.dma_start(
                    vt_c[:],
                    d["vT"][:, st * 128:(st + 1) * 128]
                    .rearrange("(k p) c -> p k c", p=128))
                for et in range(2):
                    ps = p1ps.tile([128, 512], F32, tag="p1", name="kvps")
                    for kc in range(8):
                        nc.tensor.matmul(
                            ps[:], vt_c[:, kc * 128:(kc + 1) * 128],
                            wv_r[kc][:, et * 512:(et + 1) * 512],
                            start=(kc == 0), stop=False)
                    nc.tensor.matmul(ps[:], ones1[0:1, 0:128],
                                     bv2d[0:1, et * 512:(et + 1) * 512],
                                     start=False, stop=True)
                    nc.scalar.copy(Vsb[st][et][:], ps[:])

            # W1: hjT (dup'd, +b1, bf16) and hiT (packed by i-parity, f32)
            hj_ps = w1ps.tile([128, N], F32, tag="hjps")
            nc.tensor.matmul(hj_ps[:], w1b[:], seT[:], start=True, stop=True)
            nc.vector.tensor_scalar(hjT[:], hj_ps[:], b1d[:, 0:1], None,
                                    ALU.add)
            hi_ps = w1ps.tile([128, NQ], F32, tag="hips")
            nc.tensor.matmul(hi_ps[:], w1a[:], seQ[:], start=True, stop=True)
            hi_v = hi_ps[:].rearrange("p (i two) -> p i two", two=2)
            nc.vector.tensor_copy(hiT[0:64, :], hi_v[0:64, :, 0])
            nc.vector.tensor_copy(hiT[64:128, :], hi_v[64:128, :, 1])
            if debug:
                nc.sync.dma_start(dbg["dQT"][:], QT[0][:].bitcast(F32))
                nc.sync.dma_start(dbg["dKT"][:], KT[0][:].bitcast(F32))
                nc.sync.dma_start(dbg["dV"][:], Vsb[0][0][:])
                nc.sync.dma_start(dbg["dhjT"][:], hjT[:])
                nc.sync.dma_start(dbg["dhiT"][:], hiT[:])

        # ---------------- phase 2: CAB pair MLP + transpose ----------------
        with tc.tile_pool(name="hpool", bufs=3) as hpool, \
             tc.tile_pool(name="h2sb", bufs=6) as h2sbp, \
             tc.tile_pool(name="csb", bufs=3) as csbp, \
             tc.tile_pool(name="h2ps", bufs=2, space="PSUM") as h2ps, \
             tc.tile_pool(name="cps", bufs=2, space="PSUM") as cps, \
             tc.tile_pool(name="trps", bufs=2, space="PSUM") as trps:

            for tt in range(NTT):
                h2_tiles = []
                for iic in range(4):
                    ii = tt * 4 + iic
                    h_t = hpool.tile([128, N], BF16, tag="h")
                    eng = nc.gpsimd if (ii % 4 == 3) else nc.vector
                    eng.tensor_scalar(h_t[:], hjT[:], hiT[:, ii:ii + 1], 0.0,
                                      ALU.add, ALU.max)
                    ps = h2ps.tile([128, N], F32, tag="h2")
                    nc.tensor.matmul(ps[:], w2bd[:], h_t[:], start=True,
                                     stop=True)
                    h2_t = h2sbp.tile([128, N], BF16, tag="h2sb")
                    if ii % 3 == 0:
                        nc.vector.tensor_scalar(h2_t[:], ps[:], b2d[:, 0:1],
                                                0.0, ALU.add, ALU.max)
                    else:
                        nc.scalar.activation(h2_t[:], ps[:], AF.Relu,
                                             bias=b2d[:, 0:1])
                    h2_tiles.append(h2_t)

                c_ps = cps.tile([128, N], F32, tag="comp")
                for iic in range(4):
                    nc.tensor.matmul(c_ps[32 * iic:32 * iic + 32, :],
                                     w3bd[:], h2_tiles[iic][:],
                                     start=True, stop=True,
                                     tile_position=(0, 32 * iic))
                c_sb = csbp.tile([128, N], BF16, tag="csb")
                if tt % 2 == 0:
                    nc.vector.tensor_scalar(c_sb[:], c_ps[:], t128[:, 0:1],
                                            b3t[:, 0:1], ALU.mult, ALU.add)
                else:
                    nc.scalar.activation(c_sb[:], c_ps[:], AF.Identity,
                                         bias=b3t[:, 0:1], scale=t128[:, 0:1])
                tr_ps = trps.tile([128, 512], BF16, tag="tr")
                for jc in range(4):
                    nc.tensor.transpose(tr_ps[:, jc * 128:(jc + 1) * 128],
                                        c_sb[:, jc * 128:(jc + 1) * 128],
                                        id128[:])
                nc.vector.tensor_copy(biasT[:, tt * 512:(tt + 1) * 512],
                                      tr_ps[:])
            if debug:
                nc.sync.dma_start(dbg["dbiasT"][:], biasT[:])

        # ---------------- phase 3: scores + softmax + AV ----------------
        with tc.tile_pool(name="attnT", bufs=8) as attp, \
             tc.tile_pool(name="scps", bufs=3, space="PSUM") as scps, \
             tc.tile_pool(name="smps", bufs=1, space="PSUM") as smps, \
             tc.tile_pool(name="avps", bufs=2, space="PSUM") as avps, \
             tc.tile_pool(name="r2sb", bufs=2) as r2sb:

            sums_ps = smps.tile([16, NQ], F32, tag="sums")
            for h in range(16):
                hp, hw = h // 2, (h % 2) * 64
                if h % 2 == 0:
                    av_ps = avps.tile([128, NQ], F32, tag="av")
                at_h = []
                for jc in range(4):
                    sc_ps = scps.tile([128, NQ], F32, tag="sc")
                    bview = biasT[:].rearrange(
                        "p (t j i m x) -> p t j i m x",
                        t=NTT, j=4, i=4, m=2, x=16)[:, :, jc, :, :, h]
                    nc.tensor.matmul(sc_ps[:], id128[:], bview,
                                     start=True, stop=False)
                    nc.tensor.matmul(
                        sc_ps[:],
                        KT[hp][hw:hw + 64, jc * 128:(jc + 1) * 128],
                        QT[hp][hw:hw + 64, :],
                        start=False, stop=True, skip_group_check=True)
                    at = attp.tile([128, NQ], BF16, tag="at")
                    nc.scalar.activation(at[:], sc_ps[:], AF.Exp)
                    if debug and h == 0 and jc == 0:
                        nc.sync.dma_start(dbg["dat"][:], at[:])
                    at_h.append(at)
                for jc in range(4):
                    nc.tensor.matmul(
                        sums_ps[:], hsel[:, h * 16:(h + 1) * 16], at_h[jc][:],
                        start=(h == 0 and jc == 0),
                        stop=(h == 15 and jc == 3), skip_group_check=True)
                for jc in range(4):
                    st, et = jc, h // 8
                    for k2 in range(2):
                        nc.tensor.matmul(
                            av_ps[hw + 32 * k2:hw + 32 * k2 + 32, :],
                            Vsb[st][et][:, (h % 8) * 64 + 32 * k2:
                                        (h % 8) * 64 + 32 * k2 + 32],
                            at_h[jc][:],
                            start=(jc == 0), stop=(jc == 3),
                            skip_group_check=True,
                            tile_position=(0, hw + 32 * k2))
                if h % 2 == 1:
                    nc.vector.tensor_copy(avU[hp][:], av_ps[:])

            nc.vector.tensor_copy(sums_sb[:], sums_ps[:])
            nc.vector.reciprocal(recip_sb[:], sums_sb[:])
            nc.sync.dma_start(rscratch[:], recip_sb[:])
            for hp in range(8):
                r2 = r2sb.tile([128, NQ], F32, tag="r2")
                rsrc = rscratch[2 * hp:2 * hp + 2, :].rearrange(
                    "h (o t) -> h o t", o=1)
                nc.sync.dma_start(r2[:], rsrc.broadcast_to([2, 64, NQ]))
                nc.vector.tensor_tensor(avN[hp][:], avU[hp][:], r2[:],
                                        ALU.mult)
            if debug:
                nc.sync.dma_start(dbg["dsums"][:], sums_sb[:])
                nc.sync.dma_start(dbg["davU"][:], avU[0][:])
                nc.sync.dma_start(dbg["davN"][:], avN[0][:])

        # ---------------- phase 4: output projection ----------------
        with tc.tile_pool(name="osb", bufs=2) as osb, \
             tc.tile_pool(name="ops", bufs=2, space="PSUM") as ops:
            for ttile in range(2):
                for et in range(2):
                    ps = ops.tile([128, 512], F32, tag="ops")
                    for hp in range(8):
                        nc.tensor.matmul(
                            ps[:], avN[hp][:, ttile * 128:(ttile + 1) * 128],
                            wo_r[hp][:, et * 512:(et + 1) * 512],
                            start=(hp == 0), stop=(hp == 7))
                    o_sb = osb.tile([128, 512], F32, tag="osb")
                    nc.scalar.copy(o_sb[:], ps[:])
                    nc.sync.dma_start(
                        out_d[ttile * 128:(ttile + 1) * 128,
                              et * 512:(et + 1) * 512], o_sb[:])

    nc.compile()
    return nc


def _host_prep(inputs):
    """Build the 8 per-core input maps from the full inputs."""
    f32 = np.float32
    q = np.ascontiguousarray(inputs["query"], f32)
    k = np.ascontiguousarray(inputs["key"], f32)
    v = np.ascontiguousarray(inputs["value"], f32)
    se = np.ascontiguousarray(inputs["state_embeddings"], f32)
    scale = f32(D) ** f32(-0.5)
    wq = np.ascontiguousarray(inputs["Wq"] * scale, f32)
    wk = np.ascontiguousarray(inputs["Wk"], f32)
    wv = np.ascontiguousarray(inputs["Wv"], f32)
    wo = np.ascontiguousarray(inputs["Wo"]).astype(_BF)
    bq = np.asarray(inputs["bq"], f32) * scale
    bk = np.asarray(inputs["bk"], f32)
    bv = np.asarray(inputs["bv"], f32)
    w1 = np.asarray(inputs["W1"], f32)
    b1 = np.asarray(inputs["b1"], f32)
    w2 = np.asarray(inputs["W2"], f32)
    b2 = np.asarray(inputs["b2"], f32)
    w3 = np.asarray(inputs["W3"], f32)
    b3 = np.asarray(inputs["b3"], f32)
    temps = np.asarray(inputs["head_temps"], f32)

    w1a_dup = np.concatenate([w1[:SD], w1[:SD]], axis=1)          # [64,128]
    w1b_dup = np.concatenate([w1[SD:], w1[SD:]], axis=1)          # [64,128]
    w2bd = np.zeros((128, 128), f32)
    w2bd[:64, :64] = w2
    w2bd[64:, 64:] = w2
    w3bd = np.zeros((128, 32), f32)
    w3bd[:64, :16] = w3          # m = h + 16*im
    w3bd[64:, 16:] = w3
    hsel = np.zeros((128, H * 16), f32)
    for h in range(H):
        hsel[:, h * 16 + h] = 1.0
    t128 = np.tile(temps, 8).reshape(128, 1)
    b3t = (np.tile(b3, 8) * np.tile(temps, 8)).reshape(128, 1)
    b1d = np.tile(b1, 2).reshape(128, 1)
    b2d = np.tile(b2, 2).reshape(128, 1)
    bq128 = bq.reshape(8, 128).T.copy()
    bk128 = bk.reshape(8, 128).T.copy()
    id128 = np.eye(128, dtype=f32).astype(_BF)
    ones1 = np.ones((1, 128), f32)
    bv2d = bv.reshape(1, E)

    shared = dict(wq=wq, wk=wk, wv=wv, wo=wo, w1a=w1a_dup, w1b=w1b_dup,
                  w2bd=w2bd.astype(_BF), w3bd=w3bd.astype(_BF),
                  id128=id128, hsel=hsel.astype(_BF), bq128=bq128, bk128=bk128,
                  b1d=b1d, b2d=b2d, t128=t128, b3t=b3t, bv2d=bv2d,
                  ones1=ones1)
    maps = []
    for c in range(NCORES):
        b, half = c // 2, c % 2
        rows = slice(half * NQ, (half + 1) * NQ)
        m = dict(shared)
        m["qT"] = np.ascontiguousarray(q[b, rows].T)
        m["kT"] = np.ascontiguousarray(k[b].T)
        m["vT"] = np.ascontiguousarray(v[b].T)
        m["seT"] = np.ascontiguousarray(se[b].T)
        m["seQ"] = np.ascontiguousarray(se[b, rows].T)
        maps.append(m)
    return maps


_cache = {}


def _get_program():
    if "nc" not in _cache:
        _cache["nc"] = _build_program()
    return _cache["nc"]


def kernel(**inputs):
    nc = _get_program()
    maps = _host_prep(inputs)
    res = run_bass_kernel_spmd(nc, maps, list(range(NCORES)))
    bo = np.asarray(inputs["bo"], np.float32)
    out = np.empty((B, N, E), np.float32)
    for c in range(NCORES):
        b, half = c // 2, c % 2
        out[b, half * NQ:(half + 1) * NQ] = res.results[c]["out"]
    return out + bo



# revision 7
# speedup vs baseline: 2.6997x; 2.6997x over previous
"""CAB multi-head attention on 8 Trainium2 NeuronCores.

Sharding: fully data-parallel, core c -> (batch b = c//2, query-half = c%2).
Each core computes 256 query rows against all 512 keys of its batch.
No collectives. Host does transposes/packing; device does all FLOPs.

Per-core layout conventions (features on partitions, tokens on free):
  QT/KT [E, t] bf16; V [s, e] bf16; scoresT/attnT [s, t] (softmax along
  partitions via one-hot-column matmuls, no max subtraction needed);
  CAB pairs i-major: h/h2 [(d, i%2), j]; comp [(iic, i%2+h), j] is
  PE-transposed into biasT [j, (tt, jc, c)] and pre-loaded into the
  scores PSUM via an identity matmul with a strided moving AP.

All matmuls bf16 (fp32 PSUM accumulate). head_temps folded into W3 on
host; b3*temps added via the exp activation bias. Projection weights
host-packed so every DMA is contiguous. Phase-1 projections interleaved
into the CAB tt loop so PE/DVE/Scalar all stay busy while weights
stream in.
"""
import sys

sys.path.insert(0, "/opt/trn_rl_repo")

import numpy as np
import ml_dtypes
from contextlib import ExitStack

import concourse.bacc as bacc
import concourse.tile as tile
from concourse import mybir
from concourse.bass_utils import run_bass_kernel_spmd

F32 = mybir.dt.float32
F32R = mybir.dt.float32r
BF16 = mybir.dt.bfloat16
AF = mybir.ActivationFunctionType
ALU = mybir.AluOpType

B, N, E, H, SD, HID = 4, 512, 1024, 16, 64, 64
D = E // H
NQ = 256            # query rows per core
NCORES = 8
NTT = NQ // 8       # 32 tt groups (4 i-pairs each) in the CAB stage

_BF = ml_dtypes.bfloat16


def _build_program():
    nc = bacc.Bacc("TRN2", target_bir_lowering=False, debug=False,
                   num_devices=NCORES)

    def din(name, shape, dt):
        return nc.dram_tensor(name, list(shape), dt, kind="ExternalInput").ap()

    d = {}
    # packed consts
    d["f32c"] = din("f32c", (128, 34), F32)      # bq(8) bk(8) b1(1) b2(1) b3t(16)
    d["bf16c"] = din("bf16c", (128, 544), BF16)  # id128 hsel w2bd w3bd
    d["w1ab"] = din("w1ab", (SD, 256), BF16)     # w1a | w1b
    d["seT"] = din("seT", (SD, N), BF16)
    d["seQ"] = din("seQ", (SD, NQ), BF16)
    d["sel8"] = din("sel8", (16, 1024), F32R)    # recip row-broadcast selectors
    d["onespk"] = din("onespk", (1, 128 + E), BF16)  # ones(128) | bv
    # per-core activations
    d["qT"] = din("qT", (E, NQ), BF16)
    d["kT"] = din("kT", (E, N), BF16)
    d["vtp"] = din("vtp", (N, E), BF16)          # packed V-proj stationary
    # weights (packed/bf16)
    d["wqp"] = din("wqp", (E, E), BF16)
    d["wkp"] = din("wkp", (E, E), BF16)
    d["wv"] = din("wv", (E, E), BF16)
    d["wo"] = din("wo", (E, E), BF16)
    out_d = nc.dram_tensor("out", [NQ, E], F32, kind="ExternalOutput").ap()

    with tile.TileContext(nc) as tc, ExitStack() as ctx:
        # ---------------- persistent SBUF ----------------
        cst = ctx.enter_context(tc.tile_pool(name="cst", bufs=1))
        big = ctx.enter_context(tc.tile_pool(name="big", bufs=1))

        def cload(name, shape, dt, src=None):
            t = cst.tile(list(shape), dt, tag=name, name=name)
            nc.sync.dma_start(t[:], (src if src is not None else d[name][:]))
            return t

        # small consts first (CAB needs them immediately)
        f32c = cload("f32c", (128, 34), F32)
        bq128 = f32c[:, 0:8]
        bk128 = f32c[:, 8:16]
        b1d = f32c[:, 16:17]
        b2d = f32c[:, 17:18]
        b3t128 = f32c[:, 18:34]
        bf16c = cload("bf16c", (128, 544), BF16)
        id128 = bf16c[:, 0:128]
        hsel = bf16c[:, 128:384]
        w2bd = bf16c[:, 384:512]
        w3bd = bf16c[:, 512:544]
        w1ab = cload("w1ab", (SD, 256), BF16)
        seQ = cload("seQ", (SD, NQ), BF16)
        seT = cload("seT", (SD, N), BF16)
        sel8 = cload("sel8", (16, 1024), F32R)
        onespk = cload("onespk", (1, 128 + E), BF16)
        ones1 = onespk[:, 0:128]
        bv2d = onespk[:, 128:128 + E]

        # big per-core inputs / weights, one contiguous DMA each, in
        # priority order (CAB first, out-proj weights last).
        def kview(name, t, chunk_elems):
            tt = big.tile([128, t], BF16, tag=name, name=name)
            nc.sync.dma_start(
                tt[:], d[name][:].rearrange("(k p) t -> p k t", p=128))
            return tt

        kTt = kview("kT", 8 * N, N)            # [128, (k s)]
        vtp = kview("vtp", 4 * E, E)           # [128, (st kc)]
        qTt = kview("qT", 8 * NQ, NQ)          # [128, (k t)]
        wv_r = kview("wv", 8 * E, E)
        wkp = kview("wkp", 8 * E, E)
        wqp = kview("wqp", 8 * E, E)
        wo_r = kview("wo", 8 * E, E)

        def kc(tbl, k, w):
            return tbl[:, k * w:(k + 1) * w]

        # persistent intermediates
        QT = big.tile([128, 8 * NQ], BF16, tag="QT")
        KT = big.tile([128, 8 * N], BF16, tag="KT")
        Vsb = big.tile([128, 8 * 512], BF16, tag="Vsb")   # (st et) chunks
        hjT = big.tile([128, N], BF16, tag="hjT")
        hiT = big.tile([128, 128], F32, tag="hiT")
        biasT = big.tile([128, NTT * 512], BF16, tag="biasT")
        avU = big.tile([128, 8 * NQ], F32, tag="avU")
        avN = big.tile([128, 8 * NQ], BF16, tag="avN")
        sums_sb = big.tile([16, NQ], F32, tag="sums_sb")
        recip_sb = big.tile([16, NQ], F32R, tag="recip_sb")

        # ---------------- phase A: CAB MLP with projections woven in ----
        with tc.tile_pool(name="hpool", bufs=6) as hpool, \
             tc.tile_pool(name="h2sb", bufs=3) as h2sbp, \
             tc.tile_pool(name="csb", bufs=3) as csbp, \
             tc.tile_pool(name="w2ps", bufs=2, space="PSUM") as w2ps, \
             tc.tile_pool(name="cps", bufs=2, space="PSUM") as cps, \
             tc.tile_pool(name="trps", bufs=1, space="PSUM") as trps, \
             tc.tile_pool(name="p1ps", bufs=1, space="PSUM") as p1ps:

            # W1: hjT (dup'd, +b1, bf16) and hiT (packed by i-parity, f32)
            hj_ps = p1ps.tile([128, N], F32, tag="p1")
            nc.tensor.matmul(hj_ps[:], w1ab[:, 128:256], seT[:],
                             start=True, stop=True)
            nc.vector.tensor_scalar(hjT[:], hj_ps[:], b1d[:, 0:1], None,
                                    ALU.add)
            hi_ps = p1ps.tile([128, N], F32, tag="p1", name="hi_ps")[:, 0:NQ]
            nc.tensor.matmul(hi_ps[:], w1ab[:, 0:128], seQ[:],
                             start=True, stop=True)
            hi_v = hi_ps[:].rearrange("p (i two) -> p i two", two=2)
            nc.vector.tensor_copy(hiT[0:64, :], hi_v[0:64, :, 0])
            nc.vector.tensor_copy(hiT[64:128, :], hi_v[64:128, :, 1])

            # phase-1 chunk emitters (interleaved into the tt loop)
            def v_chunk(i):
                st, et = i // 2, i % 2
                ps = p1ps.tile([128, N], F32, tag="p1")
                for k in range(8):
                    nc.tensor.matmul(
                        ps[:], kc(vtp, st, E)[:, k * 128:(k + 1) * 128],
                        kc(wv_r, k, E)[:, et * 512:(et + 1) * 512],
                        start=(k == 0), stop=False)
                nc.tensor.matmul(ps[:], ones1[0:1, 0:128],
                                 bv2d[0:1, et * 512:(et + 1) * 512],
                                 start=False, stop=True)
                nc.scalar.copy(Vsb[:, (st * 2 + et) * 512:
                                   (st * 2 + et + 1) * 512], ps[:])

            def k_chunk(ec):
                ps = p1ps.tile([128, N], F32, tag="p1")
                for k in range(8):
                    nc.tensor.matmul(
                        ps[:], kc(wkp, ec, E)[:, k * 128:(k + 1) * 128],
                        kc(kTt, k, N), start=(k == 0), stop=(k == 7))
                nc.vector.tensor_scalar(kc(KT, ec, N), ps[:],
                                        bk128[:, ec:ec + 1], None, ALU.add)

            def q_chunk(ec):
                ps = p1ps.tile([128, N], F32, tag="p1", name="qps")[:, 0:NQ]
                for k in range(8):
                    nc.tensor.matmul(
                        ps[:], kc(wqp, ec, E)[:, k * 128:(k + 1) * 128],
                        kc(qTt, k, NQ), start=(k == 0), stop=(k == 7))
                nc.vector.tensor_scalar(kc(QT, ec, NQ), ps[:],
                                        bq128[:, ec:ec + 1], None, ALU.add)

            sched = {}
            for n, i in enumerate(range(8)):          # V at tt 2..9
                sched.setdefault(2 + n, []).append(("v", i))
            for n, ec in enumerate(range(8)):         # K at tt 11,13..25
                sched.setdefault(11 + 2 * n, []).append(("k", ec))
            for n, ec in enumerate(range(8)):         # Q at tt 24..31
                sched.setdefault(24 + n, []).append(("q", ec))

            for tt in range(NTT):
                h_tiles = []
                for iic in range(4):
                    ii = tt * 4 + iic
                    h_t = hpool.tile([128, N], BF16, tag="h")
                    nc.vector.tensor_scalar(h_t[:], hjT[:],
                                            hiT[:, ii:ii + 1], 0.0,
                                            ALU.add, ALU.max)
                    h_tiles.append(h_t)
                h2_tiles = []
                for half in range(2):
                    ps2 = w2ps.tile([128, 2 * N], F32, tag="h2")
                    for j in range(2):
                        nc.tensor.matmul(ps2[:, j * N:(j + 1) * N], w2bd[:],
                                         h_tiles[half * 2 + j][:],
                                         start=True, stop=True)
                    h2_t = h2sbp.tile([128, 2 * N], BF16, tag="h2sb")
                    nc.scalar.activation(h2_t[:], ps2[:], AF.Relu,
                                         bias=b2d[:, 0:1])
                    h2_tiles.append(h2_t)

                c_ps = cps.tile([128, N], F32, tag="comp")
                for iic in range(4):
                    nc.tensor.matmul(c_ps[32 * iic:32 * iic + 32, :],
                                     w3bd[:],
                                     h2_tiles[iic // 2][:, (iic % 2) * N:
                                                        (iic % 2 + 1) * N],
                                     start=True, stop=True,
                                     tile_position=(0, 32 * iic))
                c_sb = csbp.tile([128, N], BF16, tag="csb")
                if tt % 2 == 0:
                    nc.vector.tensor_copy(c_sb[:], c_ps[:])
                else:
                    nc.scalar.copy(c_sb[:], c_ps[:])
                tr_ps = trps.tile([128, 512], BF16, tag="tr")
                for jc in range(4):
                    nc.tensor.transpose(tr_ps[:, jc * 128:(jc + 1) * 128],
                                        c_sb[:, jc * 128:(jc + 1) * 128],
                                        id128[:])
                nc.vector.tensor_copy(biasT[:, tt * 512:(tt + 1) * 512],
                                      tr_ps[:])

                for kind, i in sched.get(tt, []):
                    (v_chunk if kind == "v" else
                     k_chunk if kind == "k" else q_chunk)(i)

        # ---------------- phase B: scores + softmax + AV ----------------
        with tc.tile_pool(name="attnT", bufs=8) as attp, \
             tc.tile_pool(name="scps", bufs=3, space="PSUM") as scps, \
             tc.tile_pool(name="smps", bufs=1, space="PSUM") as smps, \
             tc.tile_pool(name="avps", bufs=2, space="PSUM") as avps, \
             tc.tile_pool(name="r2ps", bufs=2, space="PSUM") as r2ps:

            sums_ps = smps.tile([16, NQ], F32, tag="sums")
            av_tiles = [None] * 8
            for h in range(16):
                hp, hw = h // 2, (h % 2) * 64
                if h % 2 == 0:
                    av_ps = avps.tile([128, NQ], F32, tag="av")
                    av_tiles[hp] = av_ps
                at_h = []
                for jc in range(4):
                    sc_ps = scps.tile([128, NQ], F32, tag="sc")
                    bview = biasT[:].rearrange(
                        "p (t j i m x) -> p t j i m x",
                        t=NTT, j=4, i=4, m=2, x=16)[:, :, jc, :, :, h]
                    nc.tensor.matmul(sc_ps[:], id128[:], bview,
                                     start=True, stop=False)
                    nc.tensor.matmul(
                        sc_ps[:],
                        kc(KT, hp, N)[hw:hw + 64, jc * 128:(jc + 1) * 128],
                        kc(QT, hp, NQ)[hw:hw + 64, :],
                        start=False, stop=True, skip_group_check=True)
                    at = attp.tile([128, NQ], BF16, tag="at")
                    nc.scalar.activation(at[:], sc_ps[:], AF.Exp,
                                         bias=b3t128[:, h:h + 1])
                    at_h.append(at)
                for jc in range(4):
                    nc.tensor.matmul(
                        sums_ps[:], hsel[:, h * 16:(h + 1) * 16], at_h[jc][:],
                        start=(h == 0 and jc == 0),
                        stop=(h == 15 and jc == 3), skip_group_check=True)
                for jc in range(4):
                    st, et = jc, h // 8
                    nc.tensor.matmul(
                        av_ps[hw:hw + 64, :],
                        Vsb[:, (st * 2 + et) * 512 + (h % 8) * 64:
                            (st * 2 + et) * 512 + (h % 8) * 64 + 64],
                        at_h[jc][:],
                        start=(jc == 0), stop=(jc == 3),
                        skip_group_check=True,
                        tile_position=(0, hw))
                if h % 2 == 1:
                    nc.vector.tensor_copy(kc(avU, hp, NQ), av_tiles[hp][:])

            nc.vector.tensor_copy(sums_sb[:], sums_ps[:])
            with nc.allow_low_precision(reason="f32r is bit-identical f32"):
                nc.vector.reciprocal(recip_sb[:], sums_sb[:])
            for hp in range(8):
                r2 = r2ps.tile([128, NQ], F32, tag="r2")
                nc.tensor.matmul(r2[:], sel8[:, hp * 128:(hp + 1) * 128],
                                 recip_sb[:],
                                 start=True, stop=True)
                nc.vector.tensor_tensor(kc(avN, hp, NQ), kc(avU, hp, NQ),
                                        r2[:], ALU.mult)

        # ---------------- phase C: output projection ----------------
        with tc.tile_pool(name="osb", bufs=2) as osb, \
             tc.tile_pool(name="ops", bufs=2, space="PSUM") as ops:
            for ttile in range(2):
                for et in range(2):
                    ps = ops.tile([128, 512], F32, tag="ops")
                    for hp in range(8):
                        nc.tensor.matmul(
                            ps[:],
                            kc(avN, hp, NQ)[:, ttile * 128:(ttile + 1) * 128],
                            kc(wo_r, hp, E)[:, et * 512:(et + 1) * 512],
                            start=(hp == 0), stop=(hp == 7))
                    o_sb = osb.tile([128, 512], F32, tag="osb")
                    if (ttile + et) % 2 == 0:
                        nc.scalar.copy(o_sb[:], ps[:])
                    else:
                        nc.vector.tensor_copy(o_sb[:], ps[:])
                    nc.sync.dma_start(
                        out_d[ttile * 128:(ttile + 1) * 128,
                              et * 512:(et + 1) * 512], o_sb[:])

    nc.compile()
    return nc


def _host_prep(inputs):
    """Build the 8 per-core input maps from the full inputs."""
    f32 = np.float32
    q = np.ascontiguousarray(inputs["query"], f32)
    k = np.ascontiguousarray(inputs["key"], f32)
    v = np.ascontiguousarray(inputs["value"], f32)
    se = np.ascontiguousarray(inputs["state_embeddings"], f32)
    scale = f32(D) ** f32(-0.5)
    wq = np.asarray(inputs["Wq"], f32) * scale
    wk = np.asarray(inputs["Wk"], f32)
    wv = np.asarray(inputs["Wv"], f32)
    wo = np.asarray(inputs["Wo"], f32)
    bq = np.asarray(inputs["bq"], f32) * scale
    bk = np.asarray(inputs["bk"], f32)
    bv = np.asarray(inputs["bv"], f32)
    w1 = np.asarray(inputs["W1"], f32)
    b1 = np.asarray(inputs["b1"], f32)
    w2 = np.asarray(inputs["W2"], f32)
    b2 = np.asarray(inputs["b2"], f32)
    w3 = np.asarray(inputs["W3"], f32)
    b3 = np.asarray(inputs["b3"], f32)
    temps = np.asarray(inputs["head_temps"], f32)

    # packed consts
    f32c = np.zeros((128, 34), f32)
    f32c[:, 0:8] = bq.reshape(8, 128).T
    f32c[:, 8:16] = bk.reshape(8, 128).T
    f32c[:, 16] = np.tile(b1, 2)
    f32c[:, 17] = np.tile(b2, 2)
    f32c[:, 18:34] = np.tile((b3 * temps)[None, :], (128, 1))

    w3t = w3 * temps[None, :]
    w2bd = np.zeros((128, 128), f32)
    w2bd[:64, :64] = w2
    w2bd[64:, 64:] = w2
    w3bd = np.zeros((128, 32), f32)
    w3bd[:64, :16] = w3t
    w3bd[64:, 16:] = w3t
    hsel = np.zeros((128, H * 16), f32)
    for h in range(H):
        hsel[:, h * 16 + h] = 1.0
    bf16c = np.zeros((128, 544), f32)
    bf16c[:, 0:128] = np.eye(128, dtype=f32)
    bf16c[:, 128:384] = hsel
    bf16c[:, 384:512] = w2bd
    bf16c[:, 512:544] = w3bd

    w1ab = np.concatenate(
        [np.tile(w1[:SD], (1, 2)), np.tile(w1[SD:], (1, 2))],
        axis=1)                                             # [64, 256]

    sel8 = np.zeros((16, 1024), f32)
    for hp in range(8):
        sel8[2 * hp, hp * 128:hp * 128 + 64] = 1.0
        sel8[2 * hp + 1, hp * 128 + 64:hp * 128 + 128] = 1.0

    onespk = np.zeros((1, 128 + E), f32)
    onespk[0, :128] = 1.0
    onespk[0, 128:] = bv

    def packw(w):  # [in, out] f32 -> [ec*128+p, k*128+c] bf16
        return np.ascontiguousarray(
            w.reshape(8, 128, 8, 128).transpose(2, 1, 0, 3)
            .reshape(E, E)).astype(_BF)

    shared = dict(f32c=f32c, bf16c=bf16c.astype(_BF), w1ab=w1ab.astype(_BF),
                  sel8=sel8, onespk=onespk.astype(_BF),
                  wqp=packw(wq), wkp=packw(wk),
                  wv=wv.astype(_BF), wo=wo.astype(_BF))
    maps = []
    for c in range(NCORES):
        b, half = c // 2, c % 2
        rows = slice(half * NQ, (half + 1) * NQ)
        m = dict(shared)
        m["qT"] = np.ascontiguousarray(q[b, rows].T).astype(_BF)
        m["kT"] = np.ascontiguousarray(k[b].T).astype(_BF)
        vT = v[b].T                                          # [E, N]
        m["vtp"] = np.ascontiguousarray(
            vT.reshape(8, 128, 4, 128).transpose(2, 1, 0, 3)
            .reshape(N, E)).astype(_BF)
        m["seT"] = np.ascontiguousarray(se[b].T).astype(_BF)
        m["seQ"] = np.ascontiguousarray(se[b, rows].T).astype(_BF)
        maps.append(m)
    return maps


_cache = {}


def _get_program():
    if "nc" not in _cache:
        _cache["nc"] = _build_program()
    return _cache["nc"]


def kernel(**inputs):
    nc = _get_program()
    maps = _host_prep(inputs)
    res = run_bass_kernel_spmd(nc, maps, list(range(NCORES)))
    bo = np.asarray(inputs["bo"], np.float32)
    out = np.empty((B, N, E), np.float32)
    for c in range(NCORES):
        b, half = c // 2, c % 2
        out[b, half * NQ:(half + 1) * NQ] = res.results[c]["out"]
    return out + bo
